# revision 7
# baseline (speedup 1.0000x reference)
"""BERT self-attention (B=4, S=2048, D=1024, H=16) on 8 Trainium2 NeuronCores.

Tensor-parallel (Megatron) over heads: core c owns heads 2c, 2c+1.
  - Wq/Wk/Wv column-sharded (128 output dims per core), Wo row-sharded.
  - Each core consumes the full x, produces a partial (8192, 1024) output;
    partials are summed on the host (the Wo contraction over d_model is
    split across cores), plus bo.

Per-core dataflow (all big matmuls in fp32r: 11-bit-mantissa inputs,
fp32 accumulate — full PE rate at free-dim >= 256):
  xT (1024, 8192) streamed in 512-token blocks
    -> Q,K (dq 128, tok 8192) dk-major   [lhsT=WqT/WkT k-tiles, rhs=xT]
    -> V (dv 128, tok 8192), PE-transposed per 128-tok tile into
       vt [tok 128, 130] = [Vh0 64 | ones | Vh1 64 | ones]
  scores.T tile [ktok 128, q 1024] = both heads' [*, 512] halves
    (row-packed K=64 matmul pair into the two PSUM banks of one tile)
  exp on ScalarE (scale=1/8 (+ mask bias per ktok partition if needed))
  ctx.T accumulation over 16 ktok tiles: lhsT=vt[:, h*65:(h+1)*65]
    (M=65: row 64 accumulates the softmax denominators for free)
  normalize: reciprocal of row 64 -> PE outer-product broadcast -> DVE mul
  out partial [tok 128, 512] = lhsT=ctxn tok-tile, rhs=WoT
"""
import sys

if "/opt/trn_rl_repo" not in sys.path:
    sys.path.insert(0, "/opt/trn_rl_repo")

import numpy as np

import concourse.bacc as bacc
import concourse.mybir as mybir
import concourse.tile as tile
from concourse.bass_utils import run_bass_kernel_spmd

DT = mybir.dt
AF = mybir.ActivationFunctionType

B, S, D, H = 4, 2048, 1024, 16
DK = D // H  # 64
NCORES = 8
HPC = H // NCORES  # heads per core = 2
DPC = HPC * DK  # output dims per core = 128
T = B * S  # 8192 tokens
TB = 512  # token block for projections
QB = 512  # query block for attention
NKT = S // 128  # 16 key tiles per sequence
NDT = D // 128  # 8 contraction tiles for projections

_cache = {}


def _build(with_mask, phase="full", nb=B):
    nc = bacc.Bacc("TRN2", target_bir_lowering=False, debug=False)
    xT_d = nc.declare_dram_parameter("xT", [D, T], DT.float32r, isOutput=False)
    wq_d = nc.declare_dram_parameter("wqT", [D, DPC], DT.float32r, isOutput=False)
    wk_d = nc.declare_dram_parameter("wkT", [D, DPC], DT.float32r, isOutput=False)
    wv_d = nc.declare_dram_parameter("wvT", [D, DPC], DT.float32r, isOutput=False)
    wo_d = nc.declare_dram_parameter("woT", [DPC, D], DT.float32r, isOutput=False)
    bq_d = nc.declare_dram_parameter("bq", [DPC, 1], DT.float32, isOutput=False)
    bk_d = nc.declare_dram_parameter("bk", [DPC, 1], DT.float32, isOutput=False)
    bv_d = nc.declare_dram_parameter("bv", [DPC, 1], DT.float32, isOutput=False)
    id_d = nc.declare_dram_parameter("ident", [128, 128], DT.float32, isOutput=False)
    if with_mask:
        mb_d = nc.declare_dram_parameter("mbias", [B, NKT, 128], DT.float32, isOutput=False)
    out_d = nc.declare_dram_parameter("out", [T, D], DT.float32, isOutput=True)
    if phase == "qkv":
        dbg_d = nc.declare_dram_parameter("dbg", [3, 128, T], DT.float32, isOutput=True)

    with tile.TileContext(nc) as tc:
        with (
            tc.tile_pool(name="cst", bufs=1) as cst,
            tc.tile_pool(name="qkv", bufs=1) as qkv,
            tc.tile_pool(name="xt", bufs=10) as xtp,
            tc.tile_pool(name="vt", bufs=32) as vtp,
            tc.tile_pool(name="es", bufs=3) as esp,
            tc.tile_pool(name="cn", bufs=3) as cnp,
            tc.tile_pool(name="os", bufs=3) as osp,
            tc.tile_pool(name="sm", bufs=3) as smp,
            tc.tile_pool(name="sps", bufs=2, space="PSUM") as sps,
            tc.tile_pool(name="cps", bufs=2, space="PSUM") as cps,
            tc.tile_pool(name="pmm", bufs=2, space="PSUM") as pmm,
        ):
            # ---- constants / weights ----
            wq = cst.tile([128, NDT, DPC], DT.float32r, tag="wq")
            wk = cst.tile([128, NDT, DPC], DT.float32r, tag="wk")
            wv = cst.tile([128, NDT, DPC], DT.float32r, tag="wv")
            nc.sync.dma_start(wq[:], wq_d.rearrange("(a p) m -> p a m", p=128))
            nc.sync.dma_start(wk[:], wk_d.rearrange("(a p) m -> p a m", p=128))
            nc.sync.dma_start(wv[:], wv_d.rearrange("(a p) m -> p a m", p=128))
            wo = cst.tile([DPC, D], DT.float32r, tag="wo")
            nc.sync.dma_start(wo[:], wo_d[:])
            bq = cst.tile([DPC, 1], DT.float32, tag="bq")
            bk = cst.tile([DPC, 1], DT.float32, tag="bk")
            bv = cst.tile([DPC, 1], DT.float32, tag="bv")
            nc.sync.dma_start(bq[:], bq_d[:])
            nc.sync.dma_start(bk[:], bk_d[:])
            nc.sync.dma_start(bv[:], bv_d[:])
            ident = cst.tile([128, 128], DT.float32, tag="ident")
            nc.sync.dma_start(ident[:], id_d[:])
            ones128 = cst.tile([128, 1], DT.float32, tag="ones128")
            nc.vector.memset(ones128[:], 1.0)
            onesc_f = cst.tile([1, 64], DT.float32, tag="onescf")
            nc.vector.memset(onesc_f[:], 1.0)
            onesc = cst.tile([1, 64], DT.float32r, tag="onesc")
            nc.vector.tensor_copy(onesc[:], onesc_f[:])
            if with_mask:
                mb = cst.tile([128, B, NKT], DT.float32, tag="mb")
                nc.sync.dma_start(mb[:], mb_d.rearrange("b a p -> p b a"))

            # persistent activations (dk/dv-major)
            q_sb = qkv.tile([128, T], DT.float32r, tag="q")
            k_sb = qkv.tile([128, T], DT.float32r, tag="k")
            v_sb = qkv.tile([128, T], DT.float32, tag="v")

            # ---- QKV projections ----
            for tb in range(T // TB):
                xts = []
                for dt_i in range(NDT):
                    xt = xtp.tile([128, TB], DT.float32r, tag="xt", name=f"xt{tb}_{dt_i}")
                    nc.sync.dma_start(
                        xt[:],
                        xT_d[dt_i * 128 : (dt_i + 1) * 128, tb * TB : (tb + 1) * TB],
                    )
                    xts.append(xt)
                for pname, w, bias, dst in (
                    ("q", wq, bq, q_sb),
                    ("k", wk, bk, k_sb),
                    ("v", wv, bv, v_sb),
                ):
                    acc = pmm.tile([128, TB], DT.float32, tag="pmm", name=f"p{pname}{tb}")
                    for dt_i in range(NDT):
                        nc.tensor.matmul(
                            acc[:],
                            w[:, dt_i, :],
                            xts[dt_i][:],
                            start=(dt_i == 0),
                            stop=(dt_i == NDT - 1),
                        )
                    nc.vector.tensor_scalar_add(
                        dst[:, tb * TB : (tb + 1) * TB], acc[:], bias[:]
                    )

            if phase == "qkv":
                for i, src in enumerate((q_sb, k_sb, v_sb)):
                    nc.sync.dma_start(dbg_d[i], src[:].bitcast(DT.float32))
            # ---- attention per batch ----
            for b in range(B if phase == "full" else (nb if phase == "attn" else 0)):
                base = b * S
                # V transpose: vt[kt] = [tok 128, 130] fp32r
                vts = []
                for kt in range(NKT):
                    vp = pmm.tile([128, 128], DT.float32, tag="pmm", name=f"vp{b}_{kt}")
                    nc.tensor.transpose(
                        vp[:], v_sb[:, base + kt * 128 : base + (kt + 1) * 128], ident[:]
                    )
                    vt = vtp.tile([128, 130], DT.float32r, tag="vt", name=f"vt{b}_{kt}")
                    nc.vector.tensor_copy(vt[:, 0:64], vp[:, 0:64])
                    nc.vector.tensor_copy(vt[:, 65:129], vp[:, 64:128])
                    nc.vector.tensor_copy(vt[:, 64:65], ones128[:])
                    nc.vector.tensor_copy(vt[:, 129:130], ones128[:])
                    vts.append(vt)

                for qb in range(S // QB):
                    qoff = base + qb * QB
                    cps_h = [
                        cps.tile([65, QB], DT.float32, tag="ctx", name=f"c{b}_{qb}_{h}")
                        for h in range(2)
                    ]
                    for kt in range(NKT):
                        sp = sps.tile([128, 2 * QB], DT.float32, tag="sps", name=f"s{b}_{qb}_{kt}")
                        for h in range(2):
                            hp = slice(h * 64, (h + 1) * 64)
                            nc.tensor.matmul(
                                sp[:, h * QB : (h + 1) * QB],
                                k_sb[hp, base + kt * 128 : base + (kt + 1) * 128],
                                q_sb[hp, qoff : qoff + QB],
                                start=True,
                                stop=True,
                            )
                        es = esp.tile([128, 2 * QB], DT.float32r, tag="es", name=f"e{b}_{qb}_{kt}")
                        ebias = mb[:, b, kt : kt + 1] if with_mask else 0.0
                        nc.scalar.activation(es[:], sp[:], AF.Exp, bias=ebias, scale=0.125)
                        for h in range(2):
                            nc.tensor.matmul(
                                cps_h[h][:],
                                vts[kt][:, h * 65 : (h + 1) * 65],
                                es[:, h * QB : (h + 1) * QB],
                                start=(kt == 0),
                                stop=(kt == NKT - 1),
                            )
                    # normalize -> ctxn [128 dv, QB] fp32r
                    ctxn = cnp.tile([128, QB], DT.float32r, tag="cn", name=f"n{b}_{qb}")
                    for h in range(2):
                        rr = smp.tile([1, QB], DT.float32r, tag="rr", name=f"r{b}_{qb}_{h}")
                        with nc.allow_low_precision(reason="softmax reciprocal fp32r"):
                            nc.vector.reciprocal(rr[:], cps_h[h][64:65, :])
                        bc = pmm.tile([64, QB], DT.float32, tag="pmm", name=f"bc{b}_{qb}_{h}")
                        nc.tensor.matmul(bc[:], onesc[:], rr[:], start=True, stop=True)
                        bcs = smp.tile([64, QB], DT.float32, tag="bcs", name=f"bs{b}_{qb}_{h}")
                        nc.vector.tensor_copy(bcs[:], bc[:])
                        with nc.allow_low_precision(reason="ctx normalize to fp32r"):
                            nc.vector.tensor_mul(
                                ctxn[h * 64 : (h + 1) * 64, :], cps_h[h][0:64, :], bcs[:]
                            )
                    # output projection for this q block
                    for tt in range(QB // 128):
                        for ob in range(2):
                            op = pmm.tile(
                                [128, 512], DT.float32, tag="pmm", name=f"o{b}_{qb}_{tt}_{ob}"
                            )
                            nc.tensor.matmul(
                                op[:],
                                ctxn[:, tt * 128 : (tt + 1) * 128],
                                wo[:, ob * 512 : (ob + 1) * 512],
                                start=True,
                                stop=True,
                            )
                            ost = osp.tile([128, 512], DT.float32, tag="os", name=f"q{b}_{qb}_{tt}_{ob}")
                            nc.vector.tensor_copy(ost[:], op[:])
                            nc.sync.dma_start(
                                out_d[
                                    qoff + tt * 128 : qoff + (tt + 1) * 128,
                                    ob * 512 : (ob + 1) * 512,
                                ],
                                ost[:],
                            )
    nc.compile()
    return nc


def kernel(
    x,
    attention_mask,
    Wq,
    bq,
    Wk,
    bk,
    Wv,
    bv,
    Wo,
    bo,
    _trace=False,
):
    x = np.ascontiguousarray(np.asarray(x, dtype=np.float32))
    mask = np.asarray(attention_mask)
    with_mask = not bool((mask != 0).all())

    if with_mask not in _cache:
        _cache[with_mask] = _build(with_mask)
    nc = _cache[with_mask]

    xT = np.ascontiguousarray(x.reshape(T, D).T)  # (D, T)
    ident = np.eye(128, dtype=np.float32)
    in_maps = []
    for c in range(NCORES):
        r = slice(c * DPC, (c + 1) * DPC)
        m = {
            "xT": xT,
            "wqT": np.ascontiguousarray(np.asarray(Wq, np.float32)[r, :].T),
            "wkT": np.ascontiguousarray(np.asarray(Wk, np.float32)[r, :].T),
            "wvT": np.ascontiguousarray(np.asarray(Wv, np.float32)[r, :].T),
            "woT": np.ascontiguousarray(np.asarray(Wo, np.float32)[:, r].T),
            "bq": np.asarray(bq, np.float32)[r].reshape(DPC, 1),
            "bk": np.asarray(bk, np.float32)[r].reshape(DPC, 1),
            "bv": np.asarray(bv, np.float32)[r].reshape(DPC, 1),
            "ident": ident,
        }
        if with_mask:
            mbias = np.where(mask == 0, np.float32(-1e30), np.float32(0.0)).astype(
                np.float32
            )
            m["mbias"] = np.ascontiguousarray(mbias.reshape(B, NKT, 128))
        in_maps.append(m)

    res = run_bass_kernel_spmd(nc, in_maps, list(range(NCORES)), trace=_trace)
    if _trace:
        kernel.last_results = res
    acc = res.results[0]["out"].astype(np.float32).copy()
    for c in range(1, NCORES):
        acc += res.results[c]["out"]
    acc += np.asarray(bo, np.float32)[None, :]
    return acc.reshape(B, S, D)


# revision 9
# speedup vs baseline: 330.1415x; 330.1415x over previous
"""BERT self-attention (B=4, S=2048, D=1024, H=16) on 8 Trainium2 NeuronCores.

Tensor-parallel (Megatron) over heads: core c owns heads 2c, 2c+1.
  - Wq/Wk/Wv column-sharded (128 output dims per core), Wo row-sharded.
  - Each core consumes the full x, produces a partial (8192, 1024) output;
    partials are summed on the host (the Wo contraction over d_model is
    split across cores), plus bo.

Per-core dataflow (all big matmuls in fp32r: 11-bit-mantissa inputs,
fp32 accumulate — full PE rate at free-dim >= 256):
  xT (1024, 8192) streamed in 512-token blocks
    -> Q,K (dq 128, tok 8192) dk-major   [lhsT=WqT/WkT k-tiles, rhs=xT]
    -> V (dv 128, tok 8192), PE-transposed per 128-tok tile into
       vt [tok 128, 130] = [Vh0 64 | ones | Vh1 64 | ones]
  scores.T tile [ktok 128, q 1024] = both heads' [*, 512] halves
    (row-packed K=64 matmul pair into the two PSUM banks of one tile)
  exp on ScalarE (scale=1/8 (+ mask bias per ktok partition if needed))
  ctx.T accumulation over 16 ktok tiles: lhsT=vt[:, h*65:(h+1)*65]
    (M=65: row 64 accumulates the softmax denominators for free)
  normalize: reciprocal of row 64 -> PE outer-product broadcast -> DVE mul
  out partial [tok 128, 512] = lhsT=ctxn tok-tile, rhs=WoT
"""
import sys

if "/opt/trn_rl_repo" not in sys.path:
    sys.path.insert(0, "/opt/trn_rl_repo")

import numpy as np

import concourse.bacc as bacc
import concourse.mybir as mybir
import concourse.tile as tile
from concourse.bass_utils import run_bass_kernel_spmd

DT = mybir.dt
AF = mybir.ActivationFunctionType

B, S, D, H = 4, 2048, 1024, 16
DK = D // H  # 64
NCORES = 8
HPC = H // NCORES  # heads per core = 2
DPC = HPC * DK  # output dims per core = 128
T = B * S  # 8192 tokens
TB = 512  # token block for projections
QB = 512  # query block for attention
NKT = S // 128  # 16 key tiles per sequence
NDT = D // 128  # 8 contraction tiles for projections

_cache = {}


def _build(with_mask, phase="full", nb=B):
    nc = bacc.Bacc("TRN2", target_bir_lowering=False, debug=False)
    xT_d = nc.declare_dram_parameter("xT", [D, T], DT.float32r, isOutput=False)
    wq_d = nc.declare_dram_parameter("wqT", [D, DPC], DT.float32r, isOutput=False)
    wk_d = nc.declare_dram_parameter("wkT", [D, DPC], DT.float32r, isOutput=False)
    wv_d = nc.declare_dram_parameter("wvT", [D, DPC], DT.float32r, isOutput=False)
    wo_d = nc.declare_dram_parameter("woT", [DPC, D], DT.float32r, isOutput=False)
    bq_d = nc.declare_dram_parameter("bq", [DPC, 1], DT.float32, isOutput=False)
    bk_d = nc.declare_dram_parameter("bk", [DPC, 1], DT.float32, isOutput=False)
    bv_d = nc.declare_dram_parameter("bv", [DPC, 1], DT.float32, isOutput=False)
    id_d = nc.declare_dram_parameter("ident", [128, 128], DT.float32, isOutput=False)
    if with_mask:
        mb_d = nc.declare_dram_parameter("mbias", [B, NKT, 128], DT.float32, isOutput=False)
    out_d = nc.declare_dram_parameter("out", [T, D], DT.float32, isOutput=True)
    if phase == "qkv":
        dbg_d = nc.declare_dram_parameter("dbg", [3, 128, T], DT.float32, isOutput=True)

    with tile.TileContext(nc) as tc:
        with (
            tc.tile_pool(name="cst", bufs=1) as cst,
            tc.tile_pool(name="qkv", bufs=1) as qkv,
            tc.tile_pool(name="xt", bufs=10) as xtp,
            tc.tile_pool(name="vt", bufs=32) as vtp,
            tc.tile_pool(name="es", bufs=3) as esp,
            tc.tile_pool(name="cn", bufs=3) as cnp,
            tc.tile_pool(name="os", bufs=3) as osp,
            tc.tile_pool(name="sm", bufs=3) as smp,
            tc.tile_pool(name="sps", bufs=2, space="PSUM") as sps,
            tc.tile_pool(name="cps", bufs=2, space="PSUM") as cps,
            tc.tile_pool(name="pmm", bufs=2, space="PSUM") as pmm,
        ):
            # ---- constants / weights ----
            wq = cst.tile([128, NDT, DPC], DT.float32r, tag="wq")
            wk = cst.tile([128, NDT, DPC], DT.float32r, tag="wk")
            wv = cst.tile([128, NDT, DPC], DT.float32r, tag="wv")
            nc.sync.dma_start(wq[:], wq_d.rearrange("(a p) m -> p a m", p=128))
            nc.sync.dma_start(wk[:], wk_d.rearrange("(a p) m -> p a m", p=128))
            nc.sync.dma_start(wv[:], wv_d.rearrange("(a p) m -> p a m", p=128))
            wo = cst.tile([DPC, D], DT.float32r, tag="wo")
            nc.sync.dma_start(wo[:], wo_d[:])
            bq = cst.tile([DPC, 1], DT.float32, tag="bq")
            bk = cst.tile([DPC, 1], DT.float32, tag="bk")
            bv = cst.tile([DPC, 1], DT.float32, tag="bv")
            nc.sync.dma_start(bq[:], bq_d[:])
            nc.sync.dma_start(bk[:], bk_d[:])
            nc.sync.dma_start(bv[:], bv_d[:])
            ident = cst.tile([128, 128], DT.float32, tag="ident")
            nc.sync.dma_start(ident[:], id_d[:])
            ones128 = cst.tile([128, 1], DT.float32, tag="ones128")
            nc.vector.memset(ones128[:], 1.0)
            onesc_f = cst.tile([1, 64], DT.float32, tag="onescf")
            nc.vector.memset(onesc_f[:], 1.0)
            onesc = cst.tile([1, 64], DT.float32r, tag="onesc")
            nc.vector.tensor_copy(onesc[:], onesc_f[:])
            if with_mask:
                mb = cst.tile([128, B, NKT], DT.float32, tag="mb")
                nc.sync.dma_start(mb[:], mb_d.rearrange("b a p -> p b a"))

            # persistent activations (dk/dv-major)
            q_sb = qkv.tile([128, T], DT.float32r, tag="q")
            k_sb = qkv.tile([128, T], DT.float32r, tag="k")
            v_sb = qkv.tile([128, T], DT.float32, tag="v")

            # ---- QKV projections ----
            for tb in range(T // TB):
                xts = []
                for dt_i in range(NDT):
                    xt = xtp.tile([128, TB], DT.float32r, tag="xt", name=f"xt{tb}_{dt_i}")
                    nc.sync.dma_start(
                        xt[:],
                        xT_d[dt_i * 128 : (dt_i + 1) * 128, tb * TB : (tb + 1) * TB],
                    )
                    xts.append(xt)
                for pname, w, bias, dst in (
                    ("q", wq, bq, q_sb),
                    ("k", wk, bk, k_sb),
                    ("v", wv, bv, v_sb),
                ):
                    acc = pmm.tile([128, TB], DT.float32, tag="pmm", name=f"p{pname}{tb}")
                    for dt_i in range(NDT):
                        nc.tensor.matmul(
                            acc[:],
                            w[:, dt_i, :],
                            xts[dt_i][:],
                            start=(dt_i == 0),
                            stop=(dt_i == NDT - 1),
                        )
                    nc.vector.tensor_scalar_add(
                        dst[:, tb * TB : (tb + 1) * TB], acc[:], bias[:]
                    )

            if phase == "qkv":
                for i, src in enumerate((q_sb, k_sb, v_sb)):
                    nc.sync.dma_start(dbg_d[i], src[:].bitcast(DT.float32))
            # ---- attention per batch ----
            for b in range(B if phase == "full" else (nb if phase == "attn" else 0)):
                base = b * S
                # V transpose: vt[kt] = [tok 128, 130] fp32r
                vts = []
                for kt in range(NKT):
                    vp = pmm.tile([128, 128], DT.float32, tag="pmm", name=f"vp{b}_{kt}")
                    nc.tensor.transpose(
                        vp[:], v_sb[:, base + kt * 128 : base + (kt + 1) * 128], ident[:]
                    )
                    vt = vtp.tile([128, 130], DT.float32r, tag="vt", name=f"vt{b}_{kt}")
                    nc.vector.tensor_copy(vt[:, 0:64], vp[:, 0:64])
                    nc.vector.tensor_copy(vt[:, 65:129], vp[:, 64:128])
                    nc.vector.tensor_copy(vt[:, 64:65], ones128[:])
                    nc.vector.tensor_copy(vt[:, 129:130], ones128[:])
                    vts.append(vt)

                for qb in range(S // QB):
                    qoff = base + qb * QB
                    cps_h = [
                        cps.tile([65, QB], DT.float32, tag="ctx", name=f"c{b}_{qb}_{h}")
                        for h in range(2)
                    ]
                    for kt in range(NKT):
                        sp = sps.tile([128, 2 * QB], DT.float32, tag="sps", name=f"s{b}_{qb}_{kt}")
                        for h in range(2):
                            hp = slice(h * 64, (h + 1) * 64)
                            nc.tensor.matmul(
                                sp[:, h * QB : (h + 1) * QB],
                                k_sb[hp, base + kt * 128 : base + (kt + 1) * 128],
                                q_sb[hp, qoff : qoff + QB],
                                start=True,
                                stop=True,
                            )
                        es = esp.tile([128, 2 * QB], DT.float32r, tag="es", name=f"e{b}_{qb}_{kt}")
                        ebias = mb[:, b, kt : kt + 1] if with_mask else 0.0
                        nc.scalar.activation(es[:], sp[:], AF.Exp, bias=ebias, scale=0.125)
                        for h in range(2):
                            nc.tensor.matmul(
                                cps_h[h][:],
                                vts[kt][:, h * 65 : (h + 1) * 65],
                                es[:, h * QB : (h + 1) * QB],
                                start=(kt == 0),
                                stop=(kt == NKT - 1),
                            )
                    # normalize -> ctxn [128 dv, QB] fp32r
                    ctxn = cnp.tile([128, QB], DT.float32r, tag="cn", name=f"n{b}_{qb}")
                    for h in range(2):
                        rr = smp.tile([1, QB], DT.float32r, tag="rr", name=f"r{b}_{qb}_{h}")
                        with nc.allow_low_precision(reason="softmax reciprocal fp32r"):
                            nc.vector.reciprocal(rr[:], cps_h[h][64:65, :])
                        bc = pmm.tile([64, QB], DT.float32, tag="pmm", name=f"bc{b}_{qb}_{h}")
                        nc.tensor.matmul(bc[:], onesc[:], rr[:], start=True, stop=True)
                        bcs = smp.tile([64, QB], DT.float32, tag="bcs", name=f"bs{b}_{qb}_{h}")
                        nc.vector.tensor_copy(bcs[:], bc[:])
                        with nc.allow_low_precision(reason="ctx normalize to fp32r"):
                            nc.vector.tensor_mul(
                                ctxn[h * 64 : (h + 1) * 64, :], cps_h[h][0:64, :], bcs[:]
                            )
                    # output projection for this q block
                    for tt in range(QB // 128):
                        for ob in range(2):
                            op = pmm.tile(
                                [128, 512], DT.float32, tag="pmm", name=f"o{b}_{qb}_{tt}_{ob}"
                            )
                            nc.tensor.matmul(
                                op[:],
                                ctxn[:, tt * 128 : (tt + 1) * 128],
                                wo[:, ob * 512 : (ob + 1) * 512],
                                start=True,
                                stop=True,
                            )
                            ost = osp.tile([128, 512], DT.float32, tag="os", name=f"q{b}_{qb}_{tt}_{ob}")
                            nc.vector.tensor_copy(ost[:], op[:])
                            nc.sync.dma_start(
                                out_d[
                                    qoff + tt * 128 : qoff + (tt + 1) * 128,
                                    ob * 512 : (ob + 1) * 512,
                                ],
                                ost[:],
                            )
    nc.compile()
    return nc


def _make_runner(nc):
    """jit-compiled shard-mapped executor over the 8 cores, no donation so
    device-resident inputs can be reused across timed calls."""
    import jax
    from jax.experimental.shard_map import shard_map
    from jax.sharding import Mesh, NamedSharding, PartitionSpec

    from concourse import bass2jax as b2j

    b2j.install_neuronx_cc_hook()
    partition_name = nc.partition_id_tensor.name if nc.partition_id_tensor else None
    in_names, out_names, out_avals = [], [], []
    for alloc in nc.m.functions[0].allocations:
        if not isinstance(alloc, mybir.MemoryLocationSet):
            continue
        name = alloc.memorylocations[0].name
        if alloc.kind == "ExternalInput":
            if name != partition_name:
                in_names.append(name)
        elif alloc.kind == "ExternalOutput":
            out_names.append(name)
            out_avals.append(
                jax.core.ShapedArray(tuple(alloc.tensor_shape), DT.np(alloc.dtype))
            )
    n_params = len(in_names)
    all_in_names = list(in_names + out_names)
    if partition_name is not None:
        all_in_names.append(partition_name)

    def _body(*args):
        operands = list(args)
        if partition_name is not None:
            operands.append(b2j.partition_id_tensor())
        outs = b2j._bass_exec_p.bind(
            *operands,
            out_avals=tuple(out_avals),
            in_names=tuple(all_in_names),
            out_names=tuple(out_names),
            lowering_input_output_aliases=(),
            sim_require_finite=True,
            sim_require_nnan=True,
            nc=nc,
        )
        return tuple(outs)

    devices = jax.devices()[:NCORES]
    mesh = Mesh(np.asarray(devices), ("core",))
    spec = PartitionSpec("core")
    n_outs = len(out_names)
    fn = jax.jit(
        shard_map(
            _body,
            mesh=mesh,
            in_specs=(spec,) * (n_params + n_outs),
            out_specs=(spec,) * n_outs,
            check_rep=False,
        ),
        keep_unused=True,
    )
    sharding = NamedSharding(mesh, spec)

    def put(in_maps):
        concat = [
            np.concatenate([np.asarray(m[name]) for m in in_maps], axis=0)
            for name in in_names
        ]
        zeros = [
            np.zeros((NCORES * a.shape[0], *a.shape[1:]), a.dtype) for a in out_avals
        ]
        return [jax.device_put(a, sharding) for a in (*concat, *zeros)]

    return fn, put, out_names, out_avals


def _in_maps(x, attention_mask, Wq, bq, Wk, bk, Wv, bv, Wo, with_mask):
    x = np.ascontiguousarray(np.asarray(x, dtype=np.float32))
    xT = np.ascontiguousarray(x.reshape(T, D).T)  # (D, T)
    ident = np.eye(128, dtype=np.float32)
    in_maps = []
    for c in range(NCORES):
        r = slice(c * DPC, (c + 1) * DPC)
        m = {
            "xT": xT,
            "wqT": np.ascontiguousarray(np.asarray(Wq, np.float32)[r, :].T),
            "wkT": np.ascontiguousarray(np.asarray(Wk, np.float32)[r, :].T),
            "wvT": np.ascontiguousarray(np.asarray(Wv, np.float32)[r, :].T),
            "woT": np.ascontiguousarray(np.asarray(Wo, np.float32)[:, r].T),
            "bq": np.asarray(bq, np.float32)[r].reshape(DPC, 1),
            "bk": np.asarray(bk, np.float32)[r].reshape(DPC, 1),
            "bv": np.asarray(bv, np.float32)[r].reshape(DPC, 1),
            "ident": ident,
        }
        if with_mask:
            mask = np.asarray(attention_mask)
            mbias = np.where(mask == 0, np.float32(-1e30), np.float32(0.0)).astype(
                np.float32
            )
            m["mbias"] = np.ascontiguousarray(mbias.reshape(B, NKT, 128))
        in_maps.append(m)
    return in_maps


def _prepare(x, attention_mask, Wq, bq, Wk, bk, Wv, bv, Wo, bo):
    """Build (cached), upload inputs, return (fn, dev_args, out_names)."""
    mask = np.asarray(attention_mask)
    with_mask = not bool((mask != 0).all())
    key = ("runner", with_mask)
    if key not in _cache:
        nc = _build(with_mask)
        _cache[key] = _make_runner(nc)
    fn, put, out_names, out_avals = _cache[key]
    dev_args = put(
        _in_maps(x, attention_mask, Wq, bq, Wk, bk, Wv, bv, Wo, with_mask)
    )
    return fn, dev_args, out_names


def kernel(x, attention_mask, Wq, bq, Wk, bk, Wv, bv, Wo, bo):
    fn, dev_args, out_names = _prepare(
        x, attention_mask, Wq, bq, Wk, bk, Wv, bv, Wo, bo
    )
    outs = fn(*dev_args)
    out_global = np.asarray(outs[out_names.index("out")])  # (8*T, D)
    acc = out_global.reshape(NCORES, T, D).sum(axis=0, dtype=np.float32)
    acc += np.asarray(bo, np.float32)[None, :]
    return acc.reshape(B, S, D)
